# revision 44
# baseline (speedup 1.0000x reference)
"""Trainium2 Bass kernel for nn_AttributeEmbeddingLayer (gnn_message_passing).

Two-phase heterogeneous GNN attention layer on 8 NeuronCores:
  phase 1: user rows attend over product embeddings (user_nbrs)
  phase 2: product rows attend over the UPDATED user embeddings (product_nbrs)

Structure exploited (see _check_structured): the K=32 neighbors of node n
under metapath m are exactly {r + 256*u}, with r = r_m(n mod 256): all 32
nodes of a residue class share one neighbor set.  Nodes are permuted so each
core owns 32 whole classes ordered (q, u); a 128-row tile = 4 classes.

Engine mapping (v2, rebuilt from a TimelineSim profile of the previous
kernel, whose DVE ran 77% busy / ScalarE 65%):
  * h = ps + pn + b is ONE packed PE matmul per (block, metapath):
    contraction rows 0..3 = class-indicator x (pn class rows in the rhs
    ring), rows 4..67 = psT x Ibc-identity.  Lands in PSUM; ScalarE tanh
    reads PSUM directly.  b is folded into the pn table, ps comes from a
    per-phase PS stack ([indicator; (src@V)^T] x 4 metapaths).
  * scores d-reduction: x-mult on DVE (2x bf16), halving tree + final
    reduce on the otherwise-idle Pool engine.
  * softmax: exp batched per 128-row block over all 4 metapaths; den via
    strided DVE reduce; normalization as att = esc * rin (DVE 4x TSP).
  * agg and aggT as two matmuls against the same (attD, ge) SBUF tiles
    (aggT = ge^T @ attD: ge row-major is a valid lhsT, no transposes).
  * sem path: psem += Wq^T srcT + Wq^T aggT + bq (rank-1 matmul), ONE
    batched tanh [64, 512] per block into a persistent semT; beta matmuls
    run post-loop so their PSUM accumulation groups stay sequential.
  * every per-iteration gather is a STATIC-offset DMA: the host ships
    per-core pre-gathered neighbor tables (prod_perm / prod_permT), and
    phase 2 reorders the AllGather result once with two 128-row indirect
    DMAs into DRAM (t5b / t6).
  * beta epilogue (out = src + sum_m beta_m agg_m) runs on Pool.
"""

import numpy as np

# ---------------------------------------------------------------- constants
N_NODES = 8192      # nodes per type (users == products == 8192)
E = 128             # embedding dim
D = 64              # attention dim
K = 32              # neighbors per (metapath, node)
M = 4               # metapaths
CORES = 8
NLOC = N_NODES // CORES          # 1024 rows per core
NB = NLOC // 128                 # 8 n-blocks of 128 rows per core
STRIDE = N_NODES // K            # 256; neighbor sets are {r + STRIDE*u}
QPC = STRIDE // CORES            # 32 residue classes per core
NBLK = M * QPC                   # 128 (m, j) neighbor-class blocks per core
PAYR = NLOC + M * NLOC * D // E  # ag payload rows (x128 wide): 1024 + 2048

VARIANT = "full"  # "full" | "tlprof"
REPEAT = 1        # emit the kernel body this many times (slope timing)


# ---------------------------------------------------------------- host math
def _phase_np(src, other, nbrs, v, x, w, b, wq, bq, q):
    """Numpy port of the reference _phase (used as fallback / verification)."""
    m, n, k = nbrs.shape
    n_other = other.shape[0]
    beta_raw = np.zeros(m, np.float32)
    H_all = np.empty((m, n, src.shape[1]), np.float32)
    baseline = np.where(np.arange(m) == 0, np.float32(-1e-9),
                        np.float32(1.0) / n_other).astype(np.float32)
    for mi in range(m):
        agg = np.empty((n, src.shape[1]), np.float32)
        CH = 1024
        for s in range(0, n, CH):
            sl = slice(s, s + CH)
            nbr = other[nbrs[mi, sl]]                      # [CH,K,E]
            ps = src[sl] @ v[mi]                          # [CH,D]
            pn = nbr @ w[mi]                              # [CH,K,D]
            h = np.tanh(ps[:, None, :] + pn + b[mi][None, None, :])
            sc = h @ x[mi, 0]                             # [CH,K]
            mx = np.maximum(sc.max(-1), baseline[mi])
            e = np.exp(sc - mx[:, None])
            den = e.sum(-1) + (n_other - k) * np.exp(baseline[mi] - mx)
            A = e / den[:, None]
            agg[sl] = np.einsum('nk,nke->ne', A, nbr)
        H = src + agg
        H_all[mi] = H
        sem = np.tanh(H @ wq[mi] + bq[mi][None, :])
        beta_raw[mi] = (sem @ q[mi, 0]).mean()
    eb = np.exp(beta_raw - beta_raw.max())
    beta = eb / eb.sum()
    return np.einsum('m,mne->ne', beta, H_all).astype(np.float32)


def _reference_np(user, product, V, X, W_p, B_p, W_q, B_q, Q,
                  user_nbrs, product_nbrs):
    user_out = _phase_np(user, product, user_nbrs,
                         V[0], X[0], W_p[0], B_p[0], W_q[0], B_q[0], Q[0])
    product_out = _phase_np(product, user_out, product_nbrs,
                            V[1], X[1], W_p[1], B_p[1], W_q[1], B_q[1], Q[1])
    return (user_out, product_out)


def _check_structured(nbrs):
    """True iff every (m, n) neighbor set is exactly {r + STRIDE*u, u=0..K-1}
    and the neighbor class depends only on the node's own class."""
    if nbrs.shape != (M, N_NODES, K):
        return False
    r = nbrs[:, :, 0] % STRIDE
    want = r[:, :, None] + STRIDE * np.arange(K, dtype=nbrs.dtype)[None, None, :]
    if not np.array_equal(np.sort(nbrs, axis=-1), np.sort(want, axis=-1)):
        return False
    rn = r.reshape(M, N_NODES // STRIDE, STRIDE)  # node n = q + 256*u
    return bool((rn == rn[:, :1, :]).all())


# class-sort permutation: sorted position s = q*32 + u  <->  node n = q + 256*u
_PERM = (np.arange(STRIDE)[:, None] + STRIDE * np.arange(K)[None, :]).reshape(-1)
_IPERM = (np.arange(N_NODES) % STRIDE) * K + np.arange(N_NODES) // STRIDE


# ---------------------------------------------------------------- device IR
_CACHE = {}


def _build_graph():
    import sys
    if "/opt/trn_rl_repo" not in sys.path:
        sys.path.insert(0, "/opt/trn_rl_repo")
    import concourse.bass as bass
    import concourse.bacc as bacc
    import concourse.mybir as mybir
    import concourse.tile as tile

    fp = mybir.dt.float32
    bf = mybir.dt.bfloat16
    i32 = mybir.dt.int32
    AF = mybir.ActivationFunctionType
    ALU = mybir.AluOpType
    AX = mybir.AxisListType

    nc = bacc.Bacc("TRN2", target_bir_lowering=False, num_devices=CORES)

    # ---------------- I/O -------------------------------------------------
    t_user = nc.dram_tensor("user_shard", [NLOC, E], fp, kind="ExternalInput")
    t_prod = nc.dram_tensor("product_shard", [NLOC, E], fp, kind="ExternalInput")
    t_userT = nc.dram_tensor("userT", [E, NLOC], bf, kind="ExternalInput")
    t_prodT = nc.dram_tensor("prodT", [E, NLOC], bf, kind="ExternalInput")
    t_pperm = nc.dram_tensor("prod_perm", [NBLK * K, E], bf, kind="ExternalInput")
    t_ppermT = nc.dram_tensor("prod_permT", [E, NBLK * K], bf,
                              kind="ExternalInput")
    t_ind4 = nc.dram_tensor("ind4", [4, M * NLOC], bf, kind="ExternalInput")
    t_ibc = nc.dram_tensor("Ibc", [D, K * D], bf, kind="ExternalInput")
    t_V = nc.dram_tensor("V_w", [2, M, E, D], fp, kind="ExternalInput")
    t_Wp = nc.dram_tensor("Wp_w", [2, M, E, D], fp, kind="ExternalInput")
    t_Wq = nc.dram_tensor("Wq_w", [2, M, E, D], fp, kind="ExternalInput")
    t_Xrep = nc.dram_tensor("Xrep", [2, 128, M * D], fp, kind="ExternalInput")
    t_Brep = nc.dram_tensor("Brep", [2, 128, M * D], fp, kind="ExternalInput")
    t_CBrep = nc.dram_tensor("CBrep", [2, 128, M], fp, kind="ExternalInput")
    t_Bq = nc.dram_tensor("Bq_w", [2, M, D], fp, kind="ExternalInput")
    t_Q = nc.dram_tensor("Q_w", [2, M, 1, D], fp, kind="ExternalInput")
    t_ipn2 = nc.dram_tensor("i_pn2", [NBLK], i32, kind="ExternalInput")
    t_ige2 = nc.dram_tensor("i_ge2", [NBLK], i32, kind="ExternalInput")

    t_uout = nc.dram_tensor("user_out_shard", [NLOC, E], fp, kind="ExternalOutput")
    t_pout = nc.dram_tensor("product_out_shard", [NLOC, E], fp,
                            kind="ExternalOutput")

    shared = "Shared" if CORES > 4 else "Local"

    with tile.TileContext(nc) as tc:
        with (
            tc.tile_pool(name="wpool", bufs=1) as wp,
            tc.tile_pool(name="spool", bufs=1) as sp,
            tc.tile_pool(name="mpool", bufs=4) as mp,
            tc.tile_pool(name="aggpool", bufs=1) as agp,
            tc.tile_pool(name="hpsum", bufs=5, space="PSUM") as hp,
            tc.tile_pool(name="gpsum", bufs=2, space="PSUM") as gp,
            tc.tile_pool(name="spsum", bufs=1, space="PSUM") as pp,
            tc.tile_pool(name="dram", bufs=1, space="DRAM") as dp,
        ):
            # ---------------- persistent weights / patterns ----------------
            ones_row = wp.tile([1, 128], bf, name="ones_row")
            nc.vector.memset(ones_row[:], 1.0)
            ones_rf = wp.tile([1, 128], fp, name="ones_rf")
            nc.vector.memset(ones_rf[:], 1.0)

            Vall = wp.tile([E, 2 * M * D], fp, name="Vall")
            nc.sync.dma_start(Vall[:].rearrange("e (g d) -> e g d", d=D),
                              t_V[:].rearrange("ph m e d -> e (ph m) d"))
            Va = wp.tile([E, 2 * M * D], bf, name="Va")
            nc.scalar.copy(Va[:], Vall[:])
            Wpf = wp.tile([E, 2 * M * D], fp, name="Wpf")
            nc.sync.dma_start(Wpf[:].rearrange("e (g d) -> e g d", d=D),
                              t_Wp[:].rearrange("ph m e d -> e (ph m) d"))
            Wpa = wp.tile([E, 2 * M * D], bf, name="Wpa")
            nc.scalar.copy(Wpa[:], Wpf[:])
            Wqf = wp.tile([E, 2 * M * D], fp, name="Wqf")
            nc.sync.dma_start(Wqf[:].rearrange("e (g d) -> e g d", d=D),
                              t_Wq[:].rearrange("ph m e d -> e (ph m) d"))
            Wqa = wp.tile([E, 2 * M * D], bf, name="Wqa")
            nc.scalar.copy(Wqa[:], Wqf[:])
            Vw, Wpb, Wqw = {}, {}, {}
            for ph in range(2):
                for m in range(M):
                    g = ph * M + m
                    Vw[ph, m] = Va[:, g * D:(g + 1) * D]
                    Wpb[ph, m] = Wpa[:, g * D:(g + 1) * D]
                    Wqw[ph, m] = Wqa[:, g * D:(g + 1) * D]
            bqr_f = wp.tile([1, 2 * M * D], fp, name="bqr_f")
            nc.sync.dma_start(
                bqr_f[:], t_Bq[:].rearrange("ph m d -> (ph m d)")
                .rearrange("(o x) -> o x", o=1))
            bq_rows = wp.tile([1, 2 * M * D], bf, name="bq_rows")
            nc.scalar.copy(bq_rows[:], bqr_f[:])
            qTf = wp.tile([D, 2 * M], fp, name="qTf")
            nc.sync.dma_start(qTf[:], t_Q[:].rearrange("ph m o d -> d (ph m o)"))
            qTa = wp.tile([D, 2 * M], bf, name="qTa")
            nc.scalar.copy(qTa[:], qTf[:])

            xf2 = wp.tile([128, 2 * M * D], fp, name="xf2")
            nc.sync.dma_start(xf2[:].rearrange("p (ph x) -> p ph x", ph=2),
                              t_Xrep[:].rearrange("ph p x -> p ph x"))
            xb2 = wp.tile([128, 2 * M * D], bf, name="xb2")
            nc.scalar.copy(xb2[:], xf2[:])
            bf2 = wp.tile([128, 2 * M * D], fp, name="bf2")
            nc.sync.dma_start(bf2[:].rearrange("p (ph x) -> p ph x", ph=2),
                              t_Brep[:].rearrange("ph p x -> p ph x"))
            cb2 = wp.tile([128, 2 * M], fp, name="cb2")
            nc.sync.dma_start(cb2[:].rearrange("p (ph x) -> p ph x", ph=2),
                              t_CBrep[:].rearrange("ph p x -> p ph x"))
            x_all, b_all, cb_all = {}, {}, {}
            for ph in range(2):
                x_all[ph] = xb2[:, ph * M * D:(ph + 1) * M * D]
                b_all[ph] = bf2[:, ph * M * D:(ph + 1) * M * D]
                cb_all[ph] = cb2[:, ph * M:(ph + 1) * M]

            ipn2 = wp.tile([NBLK, 1], i32, name="ipn2")
            nc.sync.dma_start(ipn2[:], t_ipn2[:].rearrange("(p o) -> p o", o=1))
            ige2 = wp.tile([NBLK, 1], i32, name="ige2")
            nc.sync.dma_start(ige2[:], t_ige2[:].rearrange("(p o) -> p o", o=1))

            # wide rhs tiles, double-buffered per node-block: 4 metapath
            # slots side by side.  rows 0..63 = Ibc (replicated per slot),
            # rows 64..67 = the 4 pn class rows per slot (ONE DMA per block)
            ring = []
            for i in range(2):
                r = wp.tile([D + 4, M * K * D], bf, name=f"rhs_{i}")
                for s in range(M):
                    nc.sync.dma_start(r[0:D, s * K * D:(s + 1) * K * D],
                                      t_ibc[:])
                ring.append(r)

            # block-diagonal attention tiles (zeros persist off-diagonal)
            attD_pool = []
            for i in range(6):
                ad = wp.tile([128, 128], bf, name=f"attD_{i}")
                nc.vector.memset(ad[:], 0.0)
                attD_pool.append(ad)

            # src tiles + transposed src (host-shipped) + PS stack per phase
            srcT, src_sb, PS, semT = {}, {}, {}, {}
            for ph, t_sT, t_src in ((0, t_userT, t_user), (1, t_prodT, t_prod)):
                st = sp.tile([E, NLOC], bf, name=f"srcT_{ph}")
                nc.sync.dma_start(st[:], t_sT[:])
                srcT[ph] = st
                src_all = sp.tile([128, NB * E], fp, name=f"srcall_{ph}")
                nc.sync.dma_start(
                    src_all[:].rearrange("p (b e) -> p b e", e=E),
                    t_src[:].rearrange("(b p) e -> p b e", p=128))
                for nb in range(NB):
                    src_sb[ph, nb] = src_all[:, nb * E:(nb + 1) * E]
                ps = sp.tile([D + 4, M * NLOC], bf, name=f"PS_{ph}")
                nc.sync.dma_start(ps[D:D + 4, :], t_ind4[:])
                PS[ph] = ps
                for m in range(M):
                    for c in range(2):
                        ppsT = pp.tile([D, 512], fp, name=f"ppsT_{ph}_{m}_{c}",
                                       tag="psem", space="PSUM")
                        nc.tensor.matmul(ppsT[:], lhsT=Vw[ph, m],
                                         rhs=st[:, c * 512:(c + 1) * 512],
                                         start=True, stop=True)
                        cpeng = nc.scalar if (m + c) % 2 else nc.vector
                        if cpeng is nc.scalar:
                            cpeng.copy(
                                ps[0:D,
                                   m * NLOC + c * 512:m * NLOC + (c + 1) * 512],
                                ppsT[:])
                        else:
                            cpeng.tensor_copy(
                                ps[0:D,
                                   m * NLOC + c * 512:m * NLOC + (c + 1) * 512],
                                ppsT[:])
                semT[ph] = sp.tile([D, M * NLOC], bf, name=f"semT_{ph}")

            # ---------------- phase-1 pn table ----------------------------
            # t5a rows (m, j) hold the class block (nbr @ Wp[0,m] + b) [K*D]
            t5a = dp.tile([NBLK, K * D], bf, name="t5a")
            permT = sp.tile([E, NBLK * K], bf, name="permT")
            nc.sync.dma_start(permT[:], t_ppermT[:])
            for m in range(M):
                pwp = hp.tile([128, 512], fp, name=f"pwp_{m}", tag="h",
                              space="PSUM")
                for t in range(8):
                    g = m * 8 + t
                    nc.tensor.matmul(
                        pwp[:, t * D:(t + 1) * D],
                        lhsT=permT[:, g * 128:(g + 1) * 128],
                        rhs=Wpb[0, m], start=True, stop=True)
                pws = mp.tile([128, 512], bf, name=f"pws_{m}", tag="pws",
                              bufs=2)
                nc.vector.tensor_tensor(
                    out=pws[:].rearrange("p (t d) -> p t d", d=D),
                    in0=pwp[:].rearrange("p (t d) -> p t d", d=D),
                    in1=b_all[0][:, m * D:(m + 1) * D][:, None, :]
                        .to_broadcast([128, 8, D]),
                    op=ALU.add)
                nc.sync.dma_start(
                    t5a[m * QPC:(m + 1) * QPC, :].rearrange(
                        "(t js) (k d) -> (js k) t d", js=4, k=K),
                    pws[:].rearrange("p (t d) -> p t d", d=D))

            # ---------------- one phase ----------------------------------
            def emit_phase(ph, sfx, t5_nb, ge_nb_src, out_dram, payload):
                agg_refs = {}
                it = 0
                for nb in range(NB):
                    # one batched pn DMA + one batched ge DMA per node-block
                    rhs = ring[nb % 2]
                    nc.sync.dma_start(
                        rhs[D:D + 4, :].rearrange("p (s x) -> p s x",
                                                  x=K * D),
                        t5_nb(nb))
                    ge_all = mp.tile([128, M * E], bf, name=f"ge_{sfx}_{nb}",
                                     tag="ge", bufs=3)
                    nc.sync.dma_start(
                        ge_all[:].rearrange("p (s e) -> p s e", e=E),
                        ge_nb_src(nb))

                    sc_all = mp.tile([128, M * K], fp, name=f"sc_{sfx}_{nb}",
                                     tag="sc", bufs=2)
                    psem = pp.tile([D, M * 128], fp, name=f"psem_{sfx}_{nb}",
                                   tag="psem", space="PSUM")
                    agg_all = agp.tile([128, M * E], bf, name=f"agga_{sfx}_{nb}",
                                       tag=f"agg{nb}")
                    aggT_all = agp.tile([128, M * E], bf,
                                        name=f"aggt_{sfx}_{nb}",
                                        tag=f"aggt{nb}")
                    agg_refs[nb] = (agg_all, aggT_all)
                    ges = {}
                    for mh in range(M // 2):
                        # two metapaths share one th pair-tile so the DVE /
                        # Pool score ops run at double width
                        thp = mp.tile([128, 2 * K * D], bf,
                                      name=f"th_{sfx}_{nb}_{mh}",
                                      tag="th", bufs=3)
                        for mi in range(2):
                            m = mh * 2 + mi
                            ges[m] = ge_all[:, m * E:(m + 1) * E]
                            lhsT = PS[ph][:, m * NLOC + nb * 128:
                                          m * NLOC + (nb + 1) * 128]
                            for qtr in range(4):
                                hps = hp.tile([128, 512], fp,
                                              name=f"h_{sfx}_{nb}_{m}_{qtr}",
                                              tag="h", space="PSUM")
                                lo = m * K * D + qtr * 512
                                nc.tensor.matmul(
                                    hps[:], lhsT=lhsT,
                                    rhs=rhs[:, lo:lo + 512],
                                    start=True, stop=True)
                                nc.scalar.activation(
                                    thp[:, mi * K * D + qtr * 512:
                                        mi * K * D + (qtr + 1) * 512],
                                    hps[:], AF.Tanh)

                        # scores for the pair: x-mult (DVE 2x), halving tree
                        # split across DVE and Pool, final reduce on DVE
                        th4 = thp[:].rearrange("p (g k d) -> p g k d", g=2,
                                               d=D)
                        xmp = x_all[ph][:, mh * 2 * D:(mh + 1) * 2 * D]
                        nc.vector.tensor_tensor(
                            out=th4, in0=th4,
                            in1=xmp[:].rearrange("p (g d) -> p g d", g=2)
                                [:, :, None, :].to_broadcast([128, 2, K, D]),
                            op=ALU.mult)
                        s2eng = nc.gpsimd if it % 2 else nc.vector
                        for dd, eng in ((32, nc.vector), (16, s2eng),
                                        (8, nc.gpsimd), (4, nc.gpsimd)):
                            eng.tensor_tensor(
                                out=th4[:, :, :, :dd], in0=th4[:, :, :, :dd],
                                in1=th4[:, :, :, dd:2 * dd], op=ALU.add)
                        nc.vector.tensor_reduce(
                            sc_all[:, mh * 2 * K:(mh + 1) * 2 * K],
                            th4[:, :, :, :4], axis=AX.X, op=ALU.add)
                        it += 1

                    # softmax pieces, batched over the 4 metapaths
                    esc = mp.tile([128, M * K], bf, name=f"esc_{sfx}_{nb}",
                                  tag="esc", bufs=2)
                    nc.scalar.activation(esc[:], sc_all[:], AF.Exp)
                    den = mp.tile([128, M], fp, name=f"den_{sfx}_{nb}",
                                  tag="den", bufs=2)
                    nc.vector.tensor_reduce(
                        den[:], esc[:].rearrange("p (m k) -> p m k", k=K),
                        axis=AX.X, op=ALU.add)
                    nc.vector.tensor_tensor(out=den[:], in0=den[:],
                                            in1=cb_all[ph][:], op=ALU.add)
                    rin = mp.tile([128, M], fp, name=f"rin_{sfx}_{nb}",
                                  tag="rin", bufs=2)
                    nc.vector.reciprocal(rin[:], den[:])
                    att = mp.tile([128, M * K], bf, name=f"att_{sfx}_{nb}",
                                  tag="att", bufs=2)
                    for m in range(M):
                        nc.vector.tensor_scalar_mul(att[:, m * K:(m + 1) * K],
                                                    esc[:, m * K:(m + 1) * K],
                                                    rin[:, m:m + 1])

                    # attention aggregation + sem path per metapath
                    for m in range(M):
                        attD = attD_pool[(nb * M + m) % 6]
                        for qi in range(4):
                            sl = slice(32 * qi, 32 * (qi + 1))
                            nc.vector.transpose(attD[sl, sl],
                                                att[sl, m * K:(m + 1) * K])
                        gpair = gp.tile([128, 256], fp,
                                        name=f"gp_{sfx}_{nb}_{m}",
                                        tag="gpair", space="PSUM")
                        nc.tensor.matmul(gpair[:, 0:E], lhsT=attD[:],
                                         rhs=ges[m], start=True, stop=True)
                        nc.tensor.matmul(gpair[:, E:2 * E], lhsT=ges[m],
                                         rhs=attD[:], start=True, stop=True)
                        nc.vector.tensor_copy(agg_all[:, m * E:(m + 1) * E],
                                              gpair[:, 0:E])
                        aggT = aggT_all[:, m * E:(m + 1) * E]
                        if m % 2:
                            nc.vector.tensor_copy(aggT, gpair[:, E:2 * E])
                        else:
                            nc.scalar.copy(aggT, gpair[:, E:2 * E])
                        g = ph * M + m
                        psl = psem[:, m * 128:(m + 1) * 128]
                        nc.tensor.matmul(psl, lhsT=Wqw[ph, m],
                                         rhs=srcT[ph][:, nb * 128:(nb + 1) * 128],
                                         start=True, stop=False)
                        nc.tensor.matmul(psl, lhsT=Wqw[ph, m], rhs=aggT,
                                         start=False, stop=False)
                        nc.tensor.matmul(psl,
                                         lhsT=bq_rows[:, g * D:(g + 1) * D],
                                         rhs=ones_row[:],
                                         start=False, stop=True)

                    # batched tanh into the persistent semT (strided slice)
                    stv = semT[ph][:].rearrange("d (m b n) -> d m b n", b=NB,
                                                n=128)
                    nc.scalar.activation(
                        stv[:, :, nb, :],
                        psem[:].rearrange("d (m n) -> d m n", n=128), AF.Tanh)

                # beta matmuls post-loop: sequential accumulation groups,
                # one rotating psum tile per metapath
                ar_in = dp.tile([1, 8], fp, name=f"ar_in_{sfx}")
                ar_out = dp.tile([1, 8], fp, name=f"ar_out_{sfx}",
                                 addr_space=shared)
                braw = mp.tile([1, 8], fp, name=f"braw_{sfx}", tag="braw")
                nc.vector.memset(braw[:], 0.0)
                for m in range(M):
                    g = ph * M + m
                    pbeta = gp.tile([128, 256], fp, name=f"pbeta_{sfx}_{m}",
                                    tag="gpair", space="PSUM")
                    for nb in range(NB):
                        nc.tensor.matmul(
                            pbeta[0:1, 0:128],
                            lhsT=qTa[:, g:g + 1],
                            rhs=semT[ph][:, m * NLOC + nb * 128:
                                         m * NLOC + (nb + 1) * 128],
                            start=(nb == 0), stop=(nb == NB - 1))
                    nc.vector.tensor_reduce(braw[:, m:m + 1],
                                            pbeta[0:1, 0:128],
                                            axis=AX.X, op=ALU.add)
                nc.vector.tensor_scalar_mul(braw[:], braw[:], 1.0 / N_NODES)
                nc.gpsimd.dma_start(ar_in[:], braw[:])
                if VARIANT == "tlprof":
                    nc.gpsimd.dma_start(ar_out[:], ar_in[:])
                else:
                    nc.gpsimd.collective_compute(
                        "AllReduce", ALU.add,
                        replica_groups=[list(range(CORES))],
                        ins=[ar_in.opt()], outs=[ar_out.opt()])
                brg = mp.tile([1, 8], fp, name=f"brg_{sfx}", tag="brg")
                nc.scalar.dma_start(brg[:], ar_out[:])
                eb = mp.tile([1, M], fp, name=f"eb_{sfx}", tag="eb")
                ebs = mp.tile([1, 1], fp, name=f"ebs_{sfx}", tag="ebs")
                nc.scalar.activation(eb[:], brg[:, :M], AF.Exp, accum_out=ebs[:])
                ebr = mp.tile([1, 1], fp, name=f"ebr_{sfx}", tag="ebr")
                nc.vector.reciprocal(ebr[:], ebs[:])
                beta = mp.tile([1, M], fp, name=f"beta_{sfx}", tag="beta")
                nc.vector.tensor_scalar_mul(beta[:], eb[:], ebr[:, :1])
                pbb = gp.tile([128, 256], fp, name=f"pbb_{sfx}", tag="gpair",
                              space="PSUM")
                nc.tensor.matmul(pbb[:, :M], lhsT=ones_rf[:], rhs=beta[:],
                                 start=True, stop=True)
                beta_bc = mp.tile([128, M], fp, name=f"bbc_{sfx}", tag="bbc")
                nc.vector.tensor_copy(beta_bc[:], pbb[:, :M])

                # ---- out = src + sum_m beta_m * agg_m (DVE/Pool alternate);
                # payload pw via beta-scaled-Wp accumulating matmuls (no
                # transpose chain)
                if payload is not None:
                    wpsc = sp.tile([E, M * M * D], bf, name=f"wpsc_{sfx}",
                                   tag="wpsc")
                    for mp_ in range(M):
                        nc.vector.tensor_scalar_mul(
                            wpsc[:, mp_ * M * D:(mp_ + 1) * M * D],
                            Wpa[:, M * D:2 * M * D],
                            beta_bc[:, mp_:mp_ + 1])
                for nb in range(NB):
                    ee = nc.vector if nb % 2 == 0 else nc.gpsimd
                    agg_all, aggT_all = agg_refs[nb]
                    tmp = mp.tile([128, M * E], bf, name=f"tmp_{sfx}_{nb}",
                                  tag="tmpt", bufs=3)
                    ee.tensor_tensor(
                        out=tmp[:].rearrange("p (m e) -> p m e", e=E),
                        in0=agg_all[:].rearrange("p (m e) -> p m e", e=E),
                        in1=beta_bc[:][:, :, None].to_broadcast([128, M, E]),
                        op=ALU.mult)
                    oagg = mp.tile([128, E], fp, name=f"oagg_{sfx}_{nb}",
                                   tag="oagg", bufs=3)
                    ee.tensor_tensor(out=tmp[:, :2 * E],
                                     in0=tmp[:, :2 * E],
                                     in1=tmp[:, 2 * E:4 * E], op=ALU.add)
                    ee.tensor_tensor(out=oagg[:], in0=tmp[:, :E],
                                     in1=tmp[:, E:2 * E], op=ALU.add)
                    out_t = mp.tile([128, E], fp, name=f"out_{sfx}_{nb}",
                                    tag="outt", bufs=3)
                    ee.tensor_tensor(out=out_t[:], in0=oagg[:],
                                     in1=src_sb[ph, nb][:], op=ALU.add)
                    nc.scalar.dma_start(out_dram[nb * 128:(nb + 1) * 128, :],
                                        out_t[:])
                    if payload is not None:
                        ag_in, ag_flat = payload
                        obf = mp.tile([128, E], bf, name=f"obf_{sfx}_{nb}",
                                      tag="obf", bufs=3)
                        ee.tensor_copy(obf[:], out_t[:])
                        nc.sync.dma_start(ag_in[nb * 128:(nb + 1) * 128, :],
                                          obf[:])
                        # pw = user_out @ Wp[1] + b, accumulated from srcT and
                        # the per-metapath aggT tiles against beta-scaled Wp
                        ppw2 = hp.tile([128, 512], fp, name=f"ppw2_{sfx}_{nb}",
                                       tag="h", space="PSUM")
                        for m in range(M):
                            psl2 = ppw2[:, m * D:(m + 1) * D]
                            nc.tensor.matmul(
                                psl2,
                                lhsT=srcT[ph][:, nb * 128:(nb + 1) * 128],
                                rhs=Wpb[1, m], start=True, stop=False)
                            for mp_ in range(M):
                                nc.tensor.matmul(
                                    psl2, lhsT=aggT_all[:, mp_ * E:(mp_ + 1) * E],
                                    rhs=wpsc[:, (mp_ * M + m) * D:
                                             (mp_ * M + m + 1) * D],
                                    start=False, stop=(mp_ == M - 1))
                        pwo = mp.tile([128, M * D], bf, name=f"pwo_{sfx}_{nb}",
                                      tag="pwo", bufs=3)
                        nc.vector.tensor_tensor(out=pwo[:], in0=ppw2[:, :M * D],
                                                in1=b_all[1][:], op=ALU.add)
                        dst = ag_flat[NLOC * E:].rearrange(
                            "(m n d) -> n m d", m=M,
                            d=D)[nb * 128:(nb + 1) * 128]
                        nc.sync.dma_start(
                            dst, pwo[:].rearrange("p (m d) -> p m d", m=M))

            for rep in range(REPEAT):
                ag_in = dp.tile([PAYR, E], bf, name=f"ag_in_{rep}")
                ag_out = dp.tile([CORES * PAYR, E], bf, name=f"ag_out_{rep}",
                                 addr_space=shared)
                agf = ag_out[:].rearrange("a b -> (a b)")
                t5bd = dp.tile([NBLK, K * D], bf, name=f"t5bd_{rep}")
                t6 = dp.tile([NBLK, K * E], bf, name=f"t6_{rep}")

                # ============= phase 1: users ============================
                t5a_j = t5a[:].rearrange("(m j) x -> j m x", m=M)
                pperm_r = t_pperm[:].rearrange("(m r) e -> r m e", m=M)

                def t5a_nb(nb):
                    return t5a_j[4 * nb:4 * nb + 4]

                def ge1_nb(nb):
                    return pperm_r[nb * 128:(nb + 1) * 128]

                emit_phase(0, f"{rep}a", t5a_nb, ge1_nb, t_uout,
                           (ag_in, ag_in[:].rearrange("a b -> (a b)")))
                if VARIANT == "tlprof":
                    nc.gpsimd.dma_start(ag_out[:PAYR, :], ag_in[:])
                else:
                    nc.gpsimd.collective_compute(
                        "AllGather", mybir.AluOpType.bypass,
                        replica_groups=[list(range(CORES))],
                        ins=[ag_in.opt()], outs=[ag_out.opt()])

                # phase-2 tables: one indirect reorder each into SBUF, then
                # a DRAM roundtrip for rearranged per-block static reads
                t5b = mp.tile([NBLK, K * D], bf, name=f"t5b_{rep}",
                              tag="t5b", bufs=2)
                nc.gpsimd.indirect_dma_start(
                    out=t5b[:], out_offset=None,
                    in_=agf.rearrange("(r x) -> r x", x=K * D),
                    in_offset=bass.IndirectOffsetOnAxis(ap=ipn2[:], axis=0))
                nc.scalar.dma_start(t5bd[:], t5b[:])
                t6sb = mp.tile([NBLK, K * E], bf, name=f"t6sb_{rep}",
                               tag="t6sb", bufs=2)
                nc.gpsimd.indirect_dma_start(
                    out=t6sb[:], out_offset=None,
                    in_=agf.rearrange("(r x) -> r x", x=K * E),
                    in_offset=bass.IndirectOffsetOnAxis(ap=ige2[:], axis=0))
                nc.sync.dma_start(t6[:], t6sb[:])

                # ============= phase 2: products =========================
                t5b_j = t5bd[:].rearrange("(m j) x -> j m x", m=M)
                t6_r = t6[:].rearrange("(m j) (u e) -> (j u) m e", m=M, e=E)

                def t5b_nb(nb):
                    return t5b_j[4 * nb:4 * nb + 4]

                def ge2_nb(nb):
                    return t6_r[nb * 128:(nb + 1) * 128]

                emit_phase(1, f"{rep}b", t5b_nb, ge2_nb, t_pout, None)

    nc.compile()
    return nc


def _get_graph():
    key = ("nc", VARIANT, REPEAT)
    if key not in _CACHE:
        _CACHE[key] = _build_graph()
    return _CACHE[key]


# ---------------------------------------------------------------- runner
def _get_runner():
    """Build (once) a cached jitted SPMD executable for the graph."""
    rkey = ("runner", VARIANT, REPEAT)
    if rkey in _CACHE:
        return _CACHE[rkey]
    import sys
    if "/opt/trn_rl_repo" not in sys.path:
        sys.path.insert(0, "/opt/trn_rl_repo")
    import jax
    import numpy as _np
    from jax.experimental.shard_map import shard_map
    from jax.sharding import Mesh, PartitionSpec
    from concourse import bass2jax, mybir

    nc = _get_graph()
    bass2jax.install_neuronx_cc_hook()
    assert nc.dbg_addr is None
    pid_name = nc.partition_id_tensor.name if nc.partition_id_tensor else None

    in_names, out_names, out_avals = [], [], []
    for alloc in nc.m.functions[0].allocations:
        if not isinstance(alloc, mybir.MemoryLocationSet):
            continue
        name = alloc.memorylocations[0].name
        if alloc.kind == "ExternalInput":
            if name != pid_name:
                in_names.append(name)
        elif alloc.kind == "ExternalOutput":
            out_names.append(name)
            out_avals.append(jax.core.ShapedArray(
                tuple(alloc.tensor_shape), mybir.dt.np(alloc.dtype)))
    n_params = len(in_names)
    all_names = in_names + out_names
    if pid_name is not None:
        all_names = all_names + [pid_name]

    def _body(*args):
        operands = list(args)
        if pid_name is not None:
            operands.append(bass2jax.partition_id_tensor())
        outs = bass2jax._bass_exec_p.bind(
            *operands, out_avals=tuple(out_avals), in_names=tuple(all_names),
            out_names=tuple(out_names), lowering_input_output_aliases=(),
            sim_require_finite=True, sim_require_nnan=True, nc=nc)
        return tuple(outs)

    devices = jax.devices()[:CORES]
    mesh = Mesh(_np.asarray(devices), ("core",))
    n_outs = len(out_names)
    in_specs = (PartitionSpec("core"),) * (n_params + n_outs)
    out_specs = (PartitionSpec("core"),) * n_outs
    donate = tuple(range(n_params, n_params + n_outs))
    sharded = jax.jit(
        shard_map(_body, mesh=mesh, in_specs=in_specs, out_specs=out_specs,
                  check_rep=False),
        donate_argnums=donate, keep_unused=True)

    runner = dict(fn=sharded, in_names=in_names, out_names=out_names,
                  out_avals=out_avals, mesh=mesh)
    _CACHE[rkey] = runner
    return runner


def _run_spmd(in_maps, timeit=0):
    """Run the SPMD graph; returns (per-core results list, best_step_ns|None)."""
    import jax
    import numpy as _np
    import time as _time
    from jax.sharding import NamedSharding, PartitionSpec

    r = _get_runner()
    fn, in_names, out_names, out_avals = \
        r["fn"], r["in_names"], r["out_names"], r["out_avals"]
    mesh = r["mesh"]

    concat_in = [_np.concatenate([_np.asarray(in_maps[c][k]) for c in range(CORES)],
                                 axis=0) for k in in_names]
    sharding = NamedSharding(mesh, PartitionSpec("core"))
    dev_in = [jax.device_put(a, sharding) for a in concat_in]

    def zeros():
        return [jax.device_put(
            _np.zeros((CORES * av.shape[0], *av.shape[1:]), av.dtype), sharding)
            for av in out_avals]

    outs = fn(*dev_in, *zeros())
    jax.block_until_ready(outs)
    best_ns = None
    if timeit:
        zs = [zeros() for _ in range(timeit)]
        for z in zs:
            jax.block_until_ready(z)
        t0 = _time.perf_counter()
        outs2 = fn(*dev_in, *zs[0])
        jax.block_until_ready(outs2)
        t_one = _time.perf_counter() - t0
        t0 = _time.perf_counter()
        many = [fn(*dev_in, *z) for z in zs[1:]]
        for o in many:
            jax.block_until_ready(o)
        t_many = _time.perf_counter() - t0
        per = t_many / (timeit - 1)
        best_ns = int(per * 1e9)
        print(f"[timing] single {t_one*1e3:.2f} ms, pipelined avg {per*1e3:.3f} ms")
        outs = many[-1]
    np_outs = [_np.asarray(o) for o in outs]
    results = [{name: np_outs[i].reshape(CORES, *out_avals[i].shape)[c]
                for i, name in enumerate(out_names)} for c in range(CORES)]
    return results, best_ns


def _make_in_maps(user, product, V, X, W_p, B_p, W_q, B_q, Q,
                  user_nbrs, product_nbrs):
    import ml_dtypes
    bfnp = ml_dtypes.bfloat16

    Xrep = np.ascontiguousarray(
        np.broadcast_to(X[:, :, 0, :][:, None, :, :], (2, 128, M, D))
        .reshape(2, 128, M * D)).astype(np.float32)
    Brep = np.ascontiguousarray(
        np.broadcast_to(B_p[:, None, :, :], (2, 128, M, D))
        .reshape(2, 128, M * D)).astype(np.float32)
    CB = np.array(
        [float((N_NODES - K) * np.exp(np.float32(-1e-9)))] +
        [float((N_NODES - K) * np.exp(np.float32(1.0) / N_NODES))] * (M - 1),
        np.float32)
    CBrep = np.ascontiguousarray(
        np.broadcast_to(CB[None, None, :], (2, 128, M))).astype(np.float32)

    user_s = user[_PERM]
    prod_s = product[_PERM]
    # neighbor class per (m, own-class) -- constant across u (checked)
    r1 = (user_nbrs[:, :STRIDE, 0] % STRIDE).astype(np.int64)     # [M, 256]
    r2 = (product_nbrs[:, :STRIDE, 0] % STRIDE).astype(np.int64)  # [M, 256]

    ind_col = np.repeat(np.eye(4, dtype=np.float32), 32, axis=1)  # [4, 128]
    ind4 = np.tile(ind_col, (1, M * NLOC // 128)).astype(bfnp)
    Ibc = np.tile(np.eye(D, dtype=np.float32), (1, K)).astype(bfnp)

    u_ar = np.arange(K)
    mm = np.arange(M)[:, None]
    in_maps = []
    for c in range(CORES):
        rows = slice(c * NLOC, (c + 1) * NLOC)
        q_own = QPC * c + np.arange(QPC)
        rc1 = r1[:, q_own]                                   # [M, 32]
        rc2 = r2[:, q_own]
        # phase-1 pre-gathered neighbor rows (m, j, u): sorted row r*K + u
        src_rows = (rc1[:, :, None] * K + u_ar[None, None, :]).reshape(-1)
        pperm = np.ascontiguousarray(prod_s[src_rows]).astype(bfnp)
        ppermT = np.ascontiguousarray(pperm.T)
        # phase-2 reorder indices into the AllGather result
        c2 = rc2 // QPC
        qq = rc2 % QPC
        i_pn2 = (c2 * (PAYR * E // (K * D)) + NLOC * E // (K * D)
                 + mm * QPC + qq).astype(np.int32).reshape(-1)
        i_ge2 = (c2 * (PAYR * E // (K * E)) + qq).astype(np.int32).reshape(-1)
        in_maps.append({
            "user_shard": user_s[rows],
            "product_shard": prod_s[rows],
            "userT": np.ascontiguousarray(user_s[rows].T.astype(bfnp)),
            "prodT": np.ascontiguousarray(prod_s[rows].T.astype(bfnp)),
            "prod_perm": pperm,
            "prod_permT": ppermT,
            "ind4": ind4,
            "Ibc": Ibc,
            "V_w": V, "Wp_w": W_p, "Wq_w": W_q,
            "Xrep": Xrep, "Brep": Brep, "CBrep": CBrep,
            "Bq_w": B_q, "Q_w": Q,
            "i_pn2": i_pn2, "i_ge2": i_ge2,
        })
    return in_maps


# ---------------------------------------------------------------- entry
def kernel(user, product, V, X, W_p, B_p, W_q, B_q, Q, user_nbrs, product_nbrs):
    user = np.asarray(user, np.float32)
    product = np.asarray(product, np.float32)
    V = np.asarray(V, np.float32)
    X = np.asarray(X, np.float32)
    W_p = np.asarray(W_p, np.float32)
    B_p = np.asarray(B_p, np.float32)
    W_q = np.asarray(W_q, np.float32)
    B_q = np.asarray(B_q, np.float32)
    Q = np.asarray(Q, np.float32)
    user_nbrs = np.asarray(user_nbrs)
    product_nbrs = np.asarray(product_nbrs)

    if not (_check_structured(user_nbrs) and _check_structured(product_nbrs)):
        # General-index fallback: same math on the host.
        return _reference_np(user, product, V, X, W_p, B_p, W_q, B_q, Q,
                             user_nbrs, product_nbrs)

    in_maps = _make_in_maps(user, product, V, X, W_p, B_p, W_q, B_q, Q,
                            user_nbrs, product_nbrs)
    results, _ = _run_spmd(in_maps)
    user_out = np.concatenate([results[c]["user_out_shard"]
                               for c in range(CORES)], axis=0)[_IPERM]
    product_out = np.concatenate([results[c]["product_out_shard"]
                                  for c in range(CORES)], axis=0)[_IPERM]
    return (user_out, product_out)


# revision 45
# speedup vs baseline: 1.0334x; 1.0334x over previous
"""Trainium2 Bass kernel for nn_AttributeEmbeddingLayer (gnn_message_passing).

Two-phase heterogeneous GNN attention layer on 8 NeuronCores:
  phase 1: user rows attend over product embeddings (user_nbrs)
  phase 2: product rows attend over the UPDATED user embeddings (product_nbrs)

Structure exploited (see _check_structured): the K=32 neighbors of node n
under metapath m are exactly {r + 256*u}, with r = r_m(n mod 256): all 32
nodes of a residue class share one neighbor set.  Nodes are permuted so each
core owns 32 whole classes ordered (q, u); a 128-row tile = 4 classes.

Engine mapping (rebuilt from TimelineSim profiles of the previous kernel,
whose DVE ran 77% busy / ScalarE 65% / Pool eaten by SWDGE descgen):
  * h = ps + pn + b is a packed PE matmul per 512-wide PSUM chunk:
    contraction rows 0..63 = (src@V)^T x Ibc-identity, rows 64..67 =
    class-indicator x pn class rows.  ScalarE tanh reads PSUM directly
    ([128, 512] chunks through 5 rotating banks keep PE/ScalarE
    pipelined).  b is folded into the pn table at build time.
  * scores d-reduction runs pairwise over two metapaths per [128, 4096]
    th tile: x-mult on DVE (2x bf16), halving tree split DVE/Pool,
    final 4-way reduce on DVE.
  * softmax: exp batched per 128-row block over all 4 metapaths; den via
    strided DVE reduce; normalization as att = esc * rin (DVE 4x TSP).
  * agg and aggT as two matmuls against the same (attD, ge) SBUF tiles
    (aggT = ge^T @ attD: ge row-major is a valid lhsT, no transposes).
  * sem path: psem += Wq^T srcT + Wq^T aggT + bq (rank-1 matmul), ONE
    batched tanh [64, 512] per block into a persistent semT; beta matmuls
    run post-loop so their PSUM accumulation groups stay sequential.
  * ALL per-block gathers are STATIC-offset DMAs, batched 4-metapaths-
    at-a-time (one pn DMA + one ge DMA per 128-row block): the host ships
    per-core pre-gathered neighbor tables (prod_perm / prod_permT); phase
    2 reorders the AllGather result once with two 128-row indirect DMAs
    plus DRAM roundtrips (t5bd / t6).
  * beta epilogue (out = src + sum_m beta_m agg_m) alternates DVE/Pool
    per block; the phase-1 pw payload is rebuilt from srcT and the kept
    aggT tiles against beta-scaled Wp copies (no transpose chain), and
    post-collective reads ride the scalar DMA queue so they cannot
    head-block the sync queue.
"""

import numpy as np

# ---------------------------------------------------------------- constants
N_NODES = 8192      # nodes per type (users == products == 8192)
E = 128             # embedding dim
D = 64              # attention dim
K = 32              # neighbors per (metapath, node)
M = 4               # metapaths
CORES = 8
NLOC = N_NODES // CORES          # 1024 rows per core
NB = NLOC // 128                 # 8 n-blocks of 128 rows per core
STRIDE = N_NODES // K            # 256; neighbor sets are {r + STRIDE*u}
QPC = STRIDE // CORES            # 32 residue classes per core
NBLK = M * QPC                   # 128 (m, j) neighbor-class blocks per core
PAYR = NLOC + M * NLOC * D // E  # ag payload rows (x128 wide): 1024 + 2048

VARIANT = "full"  # "full" | "tlprof"
REPEAT = 1        # emit the kernel body this many times (slope timing)


# ---------------------------------------------------------------- host math
def _phase_np(src, other, nbrs, v, x, w, b, wq, bq, q):
    """Numpy port of the reference _phase (used as fallback / verification)."""
    m, n, k = nbrs.shape
    n_other = other.shape[0]
    beta_raw = np.zeros(m, np.float32)
    H_all = np.empty((m, n, src.shape[1]), np.float32)
    baseline = np.where(np.arange(m) == 0, np.float32(-1e-9),
                        np.float32(1.0) / n_other).astype(np.float32)
    for mi in range(m):
        agg = np.empty((n, src.shape[1]), np.float32)
        CH = 1024
        for s in range(0, n, CH):
            sl = slice(s, s + CH)
            nbr = other[nbrs[mi, sl]]                      # [CH,K,E]
            ps = src[sl] @ v[mi]                          # [CH,D]
            pn = nbr @ w[mi]                              # [CH,K,D]
            h = np.tanh(ps[:, None, :] + pn + b[mi][None, None, :])
            sc = h @ x[mi, 0]                             # [CH,K]
            mx = np.maximum(sc.max(-1), baseline[mi])
            e = np.exp(sc - mx[:, None])
            den = e.sum(-1) + (n_other - k) * np.exp(baseline[mi] - mx)
            A = e / den[:, None]
            agg[sl] = np.einsum('nk,nke->ne', A, nbr)
        H = src + agg
        H_all[mi] = H
        sem = np.tanh(H @ wq[mi] + bq[mi][None, :])
        beta_raw[mi] = (sem @ q[mi, 0]).mean()
    eb = np.exp(beta_raw - beta_raw.max())
    beta = eb / eb.sum()
    return np.einsum('m,mne->ne', beta, H_all).astype(np.float32)


def _reference_np(user, product, V, X, W_p, B_p, W_q, B_q, Q,
                  user_nbrs, product_nbrs):
    user_out = _phase_np(user, product, user_nbrs,
                         V[0], X[0], W_p[0], B_p[0], W_q[0], B_q[0], Q[0])
    product_out = _phase_np(product, user_out, product_nbrs,
                            V[1], X[1], W_p[1], B_p[1], W_q[1], B_q[1], Q[1])
    return (user_out, product_out)


def _check_structured(nbrs):
    """True iff every (m, n) neighbor set is exactly {r + STRIDE*u, u=0..K-1}
    and the neighbor class depends only on the node's own class."""
    if nbrs.shape != (M, N_NODES, K):
        return False
    r = nbrs[:, :, 0] % STRIDE
    want = r[:, :, None] + STRIDE * np.arange(K, dtype=nbrs.dtype)[None, None, :]
    if not np.array_equal(np.sort(nbrs, axis=-1), np.sort(want, axis=-1)):
        return False
    rn = r.reshape(M, N_NODES // STRIDE, STRIDE)  # node n = q + 256*u
    return bool((rn == rn[:, :1, :]).all())


# class-sort permutation: sorted position s = q*32 + u  <->  node n = q + 256*u
_PERM = (np.arange(STRIDE)[:, None] + STRIDE * np.arange(K)[None, :]).reshape(-1)
_IPERM = (np.arange(N_NODES) % STRIDE) * K + np.arange(N_NODES) // STRIDE


# ---------------------------------------------------------------- device IR
_CACHE = {}


def _build_graph():
    import sys
    if "/opt/trn_rl_repo" not in sys.path:
        sys.path.insert(0, "/opt/trn_rl_repo")
    import concourse.bass as bass
    import concourse.bacc as bacc
    import concourse.mybir as mybir
    import concourse.tile as tile

    fp = mybir.dt.float32
    bf = mybir.dt.bfloat16
    i32 = mybir.dt.int32
    AF = mybir.ActivationFunctionType
    ALU = mybir.AluOpType
    AX = mybir.AxisListType

    nc = bacc.Bacc("TRN2", target_bir_lowering=False, num_devices=CORES)

    # ---------------- I/O -------------------------------------------------
    t_user = nc.dram_tensor("user_shard", [NLOC, E], fp, kind="ExternalInput")
    t_prod = nc.dram_tensor("product_shard", [NLOC, E], fp, kind="ExternalInput")
    t_userT = nc.dram_tensor("userT", [E, NLOC], bf, kind="ExternalInput")
    t_prodT = nc.dram_tensor("prodT", [E, NLOC], bf, kind="ExternalInput")
    t_pperm = nc.dram_tensor("prod_perm", [NBLK * K, E], bf, kind="ExternalInput")
    t_ppermT = nc.dram_tensor("prod_permT", [E, NBLK * K], bf,
                              kind="ExternalInput")
    t_ind4 = nc.dram_tensor("ind4", [4, M * NLOC], bf, kind="ExternalInput")
    t_ibc = nc.dram_tensor("Ibc", [D, K * D], bf, kind="ExternalInput")
    t_V = nc.dram_tensor("V_w", [2, M, E, D], fp, kind="ExternalInput")
    t_Wp = nc.dram_tensor("Wp_w", [2, M, E, D], fp, kind="ExternalInput")
    t_Wq = nc.dram_tensor("Wq_w", [2, M, E, D], fp, kind="ExternalInput")
    t_Xrep = nc.dram_tensor("Xrep", [2, 128, M * D], fp, kind="ExternalInput")
    t_Brep = nc.dram_tensor("Brep", [2, 128, M * D], fp, kind="ExternalInput")
    t_CBrep = nc.dram_tensor("CBrep", [2, 128, M], fp, kind="ExternalInput")
    t_Bq = nc.dram_tensor("Bq_w", [2, M, D], fp, kind="ExternalInput")
    t_Q = nc.dram_tensor("Q_w", [2, M, 1, D], fp, kind="ExternalInput")
    t_ipn2 = nc.dram_tensor("i_pn2", [NBLK], i32, kind="ExternalInput")
    t_ige2 = nc.dram_tensor("i_ge2", [NBLK], i32, kind="ExternalInput")

    t_uout = nc.dram_tensor("user_out_shard", [NLOC, E], fp, kind="ExternalOutput")
    t_pout = nc.dram_tensor("product_out_shard", [NLOC, E], fp,
                            kind="ExternalOutput")

    shared = "Shared" if CORES > 4 else "Local"

    with tile.TileContext(nc) as tc:
        with (
            tc.tile_pool(name="wpool", bufs=1) as wp,
            tc.tile_pool(name="spool", bufs=1) as sp,
            tc.tile_pool(name="mpool", bufs=4) as mp,
            tc.tile_pool(name="aggpool", bufs=1) as agp,
            tc.tile_pool(name="hpsum", bufs=5, space="PSUM") as hp,
            tc.tile_pool(name="gpsum", bufs=2, space="PSUM") as gp,
            tc.tile_pool(name="spsum", bufs=1, space="PSUM") as pp,
            tc.tile_pool(name="dram", bufs=1, space="DRAM") as dp,
        ):
            # ---------------- persistent weights / patterns ----------------
            ones_row = wp.tile([1, 128], bf, name="ones_row")
            nc.vector.memset(ones_row[:], 1.0)
            ones_rf = wp.tile([1, 128], fp, name="ones_rf")
            nc.vector.memset(ones_rf[:], 1.0)

            Vall = wp.tile([E, 2 * M * D], fp, name="Vall")
            nc.sync.dma_start(Vall[:].rearrange("e (g d) -> e g d", d=D),
                              t_V[:].rearrange("ph m e d -> e (ph m) d"))
            Va = wp.tile([E, 2 * M * D], bf, name="Va")
            nc.scalar.copy(Va[:], Vall[:])
            Wpf = wp.tile([E, 2 * M * D], fp, name="Wpf")
            nc.sync.dma_start(Wpf[:].rearrange("e (g d) -> e g d", d=D),
                              t_Wp[:].rearrange("ph m e d -> e (ph m) d"))
            Wpa = wp.tile([E, 2 * M * D], bf, name="Wpa")
            nc.scalar.copy(Wpa[:], Wpf[:])
            Wqf = wp.tile([E, 2 * M * D], fp, name="Wqf")
            nc.sync.dma_start(Wqf[:].rearrange("e (g d) -> e g d", d=D),
                              t_Wq[:].rearrange("ph m e d -> e (ph m) d"))
            Wqa = wp.tile([E, 2 * M * D], bf, name="Wqa")
            nc.scalar.copy(Wqa[:], Wqf[:])
            Vw, Wpb, Wqw = {}, {}, {}
            for ph in range(2):
                for m in range(M):
                    g = ph * M + m
                    Vw[ph, m] = Va[:, g * D:(g + 1) * D]
                    Wpb[ph, m] = Wpa[:, g * D:(g + 1) * D]
                    Wqw[ph, m] = Wqa[:, g * D:(g + 1) * D]
            bqr_f = wp.tile([1, 2 * M * D], fp, name="bqr_f")
            nc.sync.dma_start(
                bqr_f[:], t_Bq[:].rearrange("ph m d -> (ph m d)")
                .rearrange("(o x) -> o x", o=1))
            bq_rows = wp.tile([1, 2 * M * D], bf, name="bq_rows")
            nc.scalar.copy(bq_rows[:], bqr_f[:])
            qTf = wp.tile([D, 2 * M], fp, name="qTf")
            nc.sync.dma_start(qTf[:], t_Q[:].rearrange("ph m o d -> d (ph m o)"))
            qTa = wp.tile([D, 2 * M], bf, name="qTa")
            nc.scalar.copy(qTa[:], qTf[:])

            xf2 = wp.tile([128, 2 * M * D], fp, name="xf2")
            nc.sync.dma_start(xf2[:].rearrange("p (ph x) -> p ph x", ph=2),
                              t_Xrep[:].rearrange("ph p x -> p ph x"))
            xb2 = wp.tile([128, 2 * M * D], bf, name="xb2")
            nc.scalar.copy(xb2[:], xf2[:])
            bf2 = wp.tile([128, 2 * M * D], fp, name="bf2")
            nc.sync.dma_start(bf2[:].rearrange("p (ph x) -> p ph x", ph=2),
                              t_Brep[:].rearrange("ph p x -> p ph x"))
            cb2 = wp.tile([128, 2 * M], fp, name="cb2")
            nc.sync.dma_start(cb2[:].rearrange("p (ph x) -> p ph x", ph=2),
                              t_CBrep[:].rearrange("ph p x -> p ph x"))
            x_all, b_all, cb_all = {}, {}, {}
            for ph in range(2):
                x_all[ph] = xb2[:, ph * M * D:(ph + 1) * M * D]
                b_all[ph] = bf2[:, ph * M * D:(ph + 1) * M * D]
                cb_all[ph] = cb2[:, ph * M:(ph + 1) * M]

            ipn2 = wp.tile([NBLK, 1], i32, name="ipn2")
            nc.sync.dma_start(ipn2[:], t_ipn2[:].rearrange("(p o) -> p o", o=1))
            ige2 = wp.tile([NBLK, 1], i32, name="ige2")
            nc.sync.dma_start(ige2[:], t_ige2[:].rearrange("(p o) -> p o", o=1))

            # wide rhs tiles, double-buffered per node-block: 4 metapath
            # slots side by side.  rows 0..63 = Ibc (replicated per slot),
            # rows 64..67 = the 4 pn class rows per slot (ONE DMA per block)
            ring = []
            for i in range(2):
                r = wp.tile([D + 4, M * K * D], bf, name=f"rhs_{i}")
                for s in range(M):
                    nc.sync.dma_start(r[0:D, s * K * D:(s + 1) * K * D],
                                      t_ibc[:])
                ring.append(r)

            # block-diagonal attention tiles (zeros persist off-diagonal)
            attD_pool = []
            for i in range(6):
                ad = wp.tile([128, 128], bf, name=f"attD_{i}")
                nc.vector.memset(ad[:], 0.0)
                attD_pool.append(ad)

            # src tiles + transposed src (host-shipped) + PS stack per phase
            srcT, src_sb, PS, semT = {}, {}, {}, {}
            for ph, t_sT, t_src in ((0, t_userT, t_user), (1, t_prodT, t_prod)):
                st = sp.tile([E, NLOC], bf, name=f"srcT_{ph}")
                nc.sync.dma_start(st[:], t_sT[:])
                srcT[ph] = st
                src_all = sp.tile([128, NB * E], fp, name=f"srcall_{ph}")
                nc.sync.dma_start(
                    src_all[:].rearrange("p (b e) -> p b e", e=E),
                    t_src[:].rearrange("(b p) e -> p b e", p=128))
                for nb in range(NB):
                    src_sb[ph, nb] = src_all[:, nb * E:(nb + 1) * E]
                ps = sp.tile([D + 4, M * NLOC], bf, name=f"PS_{ph}")
                nc.sync.dma_start(ps[D:D + 4, :], t_ind4[:])
                PS[ph] = ps
                for m in range(M):
                    for c in range(2):
                        ppsT = pp.tile([D, 512], fp, name=f"ppsT_{ph}_{m}_{c}",
                                       tag="psem", space="PSUM")
                        nc.tensor.matmul(ppsT[:], lhsT=Vw[ph, m],
                                         rhs=st[:, c * 512:(c + 1) * 512],
                                         start=True, stop=True)
                        cpeng = nc.scalar if (m + c) % 2 else nc.vector
                        if cpeng is nc.scalar:
                            cpeng.copy(
                                ps[0:D,
                                   m * NLOC + c * 512:m * NLOC + (c + 1) * 512],
                                ppsT[:])
                        else:
                            cpeng.tensor_copy(
                                ps[0:D,
                                   m * NLOC + c * 512:m * NLOC + (c + 1) * 512],
                                ppsT[:])
                semT[ph] = sp.tile([D, M * NLOC], bf, name=f"semT_{ph}")

            # ---------------- phase-1 pn table ----------------------------
            # t5a rows (m, j) hold the class block (nbr @ Wp[0,m] + b) [K*D]
            t5a = dp.tile([NBLK, K * D], bf, name="t5a")
            permT = sp.tile([E, NBLK * K], bf, name="permT")
            nc.sync.dma_start(permT[:], t_ppermT[:])
            for m in range(M):
                pwp = hp.tile([128, 512], fp, name=f"pwp_{m}", tag="h",
                              space="PSUM")
                for t in range(8):
                    g = m * 8 + t
                    nc.tensor.matmul(
                        pwp[:, t * D:(t + 1) * D],
                        lhsT=permT[:, g * 128:(g + 1) * 128],
                        rhs=Wpb[0, m], start=True, stop=True)
                pws = mp.tile([128, 512], bf, name=f"pws_{m}", tag="pws",
                              bufs=2)
                nc.vector.tensor_tensor(
                    out=pws[:].rearrange("p (t d) -> p t d", d=D),
                    in0=pwp[:].rearrange("p (t d) -> p t d", d=D),
                    in1=b_all[0][:, m * D:(m + 1) * D][:, None, :]
                        .to_broadcast([128, 8, D]),
                    op=ALU.add)
                nc.sync.dma_start(
                    t5a[m * QPC:(m + 1) * QPC, :].rearrange(
                        "(t js) (k d) -> (js k) t d", js=4, k=K),
                    pws[:].rearrange("p (t d) -> p t d", d=D))

            # ---------------- one phase ----------------------------------
            def emit_phase(ph, sfx, t5_nb, ge_nb_src, out_dram, payload):
                agg_refs = {}
                it = 0
                for nb in range(NB):
                    # one batched pn DMA + one batched ge DMA per node-block
                    rhs = ring[nb % 2]
                    nc.sync.dma_start(
                        rhs[D:D + 4, :].rearrange("p (s x) -> p s x",
                                                  x=K * D),
                        t5_nb(nb))
                    ge_all = mp.tile([128, M * E], bf, name=f"ge_{sfx}_{nb}",
                                     tag="ge", bufs=3)
                    nc.sync.dma_start(
                        ge_all[:].rearrange("p (s e) -> p s e", e=E),
                        ge_nb_src(nb))

                    sc_all = mp.tile([128, M * K], fp, name=f"sc_{sfx}_{nb}",
                                     tag="sc", bufs=2)
                    psem = pp.tile([D, M * 128], fp, name=f"psem_{sfx}_{nb}",
                                   tag="psem", space="PSUM")
                    agg_all = agp.tile([128, M * E], bf, name=f"agga_{sfx}_{nb}",
                                       tag=f"agg{nb}")
                    aggT_all = agp.tile([128, M * E], bf,
                                        name=f"aggt_{sfx}_{nb}",
                                        tag=f"aggt{nb}")
                    agg_refs[nb] = (agg_all, aggT_all)
                    ges = {}
                    for mh in range(M // 2):
                        # two metapaths share one th pair-tile so the DVE /
                        # Pool score ops run at double width
                        thp = mp.tile([128, 2 * K * D], bf,
                                      name=f"th_{sfx}_{nb}_{mh}",
                                      tag="th", bufs=3)
                        for mi in range(2):
                            m = mh * 2 + mi
                            ges[m] = ge_all[:, m * E:(m + 1) * E]
                            lhsT = PS[ph][:, m * NLOC + nb * 128:
                                          m * NLOC + (nb + 1) * 128]
                            for qtr in range(4):
                                hps = hp.tile([128, 512], fp,
                                              name=f"h_{sfx}_{nb}_{m}_{qtr}",
                                              tag="h", space="PSUM")
                                lo = m * K * D + qtr * 512
                                nc.tensor.matmul(
                                    hps[:], lhsT=lhsT,
                                    rhs=rhs[:, lo:lo + 512],
                                    start=True, stop=True)
                                nc.scalar.activation(
                                    thp[:, mi * K * D + qtr * 512:
                                        mi * K * D + (qtr + 1) * 512],
                                    hps[:], AF.Tanh)

                        # scores for the pair: x-mult (DVE 2x), halving tree
                        # split across DVE and Pool, final reduce on DVE
                        th4 = thp[:].rearrange("p (g k d) -> p g k d", g=2,
                                               d=D)
                        xmp = x_all[ph][:, mh * 2 * D:(mh + 1) * 2 * D]
                        nc.vector.tensor_tensor(
                            out=th4, in0=th4,
                            in1=xmp[:].rearrange("p (g d) -> p g d", g=2)
                                [:, :, None, :].to_broadcast([128, 2, K, D]),
                            op=ALU.mult)
                        s2eng = nc.gpsimd if it % 2 else nc.vector
                        for dd, eng in ((32, nc.vector), (16, s2eng),
                                        (8, nc.gpsimd), (4, nc.gpsimd)):
                            eng.tensor_tensor(
                                out=th4[:, :, :, :dd], in0=th4[:, :, :, :dd],
                                in1=th4[:, :, :, dd:2 * dd], op=ALU.add)
                        nc.vector.tensor_reduce(
                            sc_all[:, mh * 2 * K:(mh + 1) * 2 * K],
                            th4[:, :, :, :4], axis=AX.X, op=ALU.add)
                        it += 1

                    # softmax pieces, batched over the 4 metapaths
                    esc = mp.tile([128, M * K], bf, name=f"esc_{sfx}_{nb}",
                                  tag="esc", bufs=2)
                    nc.scalar.activation(esc[:], sc_all[:], AF.Exp)
                    den = mp.tile([128, M], fp, name=f"den_{sfx}_{nb}",
                                  tag="den", bufs=2)
                    nc.vector.tensor_reduce(
                        den[:], esc[:].rearrange("p (m k) -> p m k", k=K),
                        axis=AX.X, op=ALU.add)
                    nc.vector.tensor_tensor(out=den[:], in0=den[:],
                                            in1=cb_all[ph][:], op=ALU.add)
                    rin = mp.tile([128, M], fp, name=f"rin_{sfx}_{nb}",
                                  tag="rin", bufs=2)
                    nc.vector.reciprocal(rin[:], den[:])
                    att = mp.tile([128, M * K], bf, name=f"att_{sfx}_{nb}",
                                  tag="att", bufs=2)
                    for m in range(M):
                        nc.vector.tensor_scalar_mul(att[:, m * K:(m + 1) * K],
                                                    esc[:, m * K:(m + 1) * K],
                                                    rin[:, m:m + 1])

                    # attention aggregation + sem path per metapath
                    for m in range(M):
                        attD = attD_pool[(nb * M + m) % 6]
                        for qi in range(4):
                            sl = slice(32 * qi, 32 * (qi + 1))
                            nc.vector.transpose(attD[sl, sl],
                                                att[sl, m * K:(m + 1) * K])
                        gpair = gp.tile([128, 256], fp,
                                        name=f"gp_{sfx}_{nb}_{m}",
                                        tag="gpair", space="PSUM")
                        nc.tensor.matmul(gpair[:, 0:E], lhsT=attD[:],
                                         rhs=ges[m], start=True, stop=True)
                        nc.tensor.matmul(gpair[:, E:2 * E], lhsT=ges[m],
                                         rhs=attD[:], start=True, stop=True)
                        nc.vector.tensor_copy(agg_all[:, m * E:(m + 1) * E],
                                              gpair[:, 0:E])
                        aggT = aggT_all[:, m * E:(m + 1) * E]
                        if m % 2:
                            nc.vector.tensor_copy(aggT, gpair[:, E:2 * E])
                        else:
                            nc.scalar.copy(aggT, gpair[:, E:2 * E])
                        g = ph * M + m
                        psl = psem[:, m * 128:(m + 1) * 128]
                        nc.tensor.matmul(psl, lhsT=Wqw[ph, m],
                                         rhs=srcT[ph][:, nb * 128:(nb + 1) * 128],
                                         start=True, stop=False)
                        nc.tensor.matmul(psl, lhsT=Wqw[ph, m], rhs=aggT,
                                         start=False, stop=False)
                        nc.tensor.matmul(psl,
                                         lhsT=bq_rows[:, g * D:(g + 1) * D],
                                         rhs=ones_row[:],
                                         start=False, stop=True)

                    # batched tanh into the persistent semT (strided slice)
                    stv = semT[ph][:].rearrange("d (m b n) -> d m b n", b=NB,
                                                n=128)
                    nc.scalar.activation(
                        stv[:, :, nb, :],
                        psem[:].rearrange("d (m n) -> d m n", n=128), AF.Tanh)

                # beta matmuls post-loop: sequential accumulation groups,
                # one rotating psum tile per metapath
                ar_in = dp.tile([1, 8], fp, name=f"ar_in_{sfx}")
                ar_out = dp.tile([1, 8], fp, name=f"ar_out_{sfx}",
                                 addr_space=shared)
                braw = mp.tile([1, 8], fp, name=f"braw_{sfx}", tag="braw")
                nc.vector.memset(braw[:], 0.0)
                for m in range(M):
                    g = ph * M + m
                    pbeta = gp.tile([128, 256], fp, name=f"pbeta_{sfx}_{m}",
                                    tag="gpair", space="PSUM")
                    for nb in range(NB):
                        nc.tensor.matmul(
                            pbeta[0:1, 0:128],
                            lhsT=qTa[:, g:g + 1],
                            rhs=semT[ph][:, m * NLOC + nb * 128:
                                         m * NLOC + (nb + 1) * 128],
                            start=(nb == 0), stop=(nb == NB - 1))
                    nc.vector.tensor_reduce(braw[:, m:m + 1],
                                            pbeta[0:1, 0:128],
                                            axis=AX.X, op=ALU.add)
                nc.vector.tensor_scalar_mul(braw[:], braw[:], 1.0 / N_NODES)
                nc.gpsimd.dma_start(ar_in[:], braw[:])
                if VARIANT == "tlprof":
                    nc.gpsimd.dma_start(ar_out[:], ar_in[:])
                else:
                    nc.gpsimd.collective_compute(
                        "AllReduce", ALU.add,
                        replica_groups=[list(range(CORES))],
                        ins=[ar_in.opt()], outs=[ar_out.opt()])
                brg = mp.tile([1, 8], fp, name=f"brg_{sfx}", tag="brg")
                nc.scalar.dma_start(brg[:], ar_out[:])
                eb = mp.tile([1, M], fp, name=f"eb_{sfx}", tag="eb")
                ebs = mp.tile([1, 1], fp, name=f"ebs_{sfx}", tag="ebs")
                nc.scalar.activation(eb[:], brg[:, :M], AF.Exp, accum_out=ebs[:])
                ebr = mp.tile([1, 1], fp, name=f"ebr_{sfx}", tag="ebr")
                nc.vector.reciprocal(ebr[:], ebs[:])
                beta = mp.tile([1, M], fp, name=f"beta_{sfx}", tag="beta")
                nc.vector.tensor_scalar_mul(beta[:], eb[:], ebr[:, :1])
                pbb = gp.tile([128, 256], fp, name=f"pbb_{sfx}", tag="gpair",
                              space="PSUM")
                nc.tensor.matmul(pbb[:, :M], lhsT=ones_rf[:], rhs=beta[:],
                                 start=True, stop=True)
                beta_bc = mp.tile([128, M], fp, name=f"bbc_{sfx}", tag="bbc")
                nc.vector.tensor_copy(beta_bc[:], pbb[:, :M])

                # ---- out = src + sum_m beta_m * agg_m (DVE/Pool alternate);
                # payload pw via beta-scaled-Wp accumulating matmuls (no
                # transpose chain)
                if payload is not None:
                    wpsc = sp.tile([E, M * M * D], bf, name=f"wpsc_{sfx}",
                                   tag="wpsc")
                    for mp_ in range(M):
                        nc.vector.tensor_scalar_mul(
                            wpsc[:, mp_ * M * D:(mp_ + 1) * M * D],
                            Wpa[:, M * D:2 * M * D],
                            beta_bc[:, mp_:mp_ + 1])
                for nb in range(NB):
                    ee = nc.vector if nb % 2 == 0 else nc.gpsimd
                    agg_all, aggT_all = agg_refs[nb]
                    tmp = mp.tile([128, M * E], bf, name=f"tmp_{sfx}_{nb}",
                                  tag="tmpt", bufs=3)
                    ee.tensor_tensor(
                        out=tmp[:].rearrange("p (m e) -> p m e", e=E),
                        in0=agg_all[:].rearrange("p (m e) -> p m e", e=E),
                        in1=beta_bc[:][:, :, None].to_broadcast([128, M, E]),
                        op=ALU.mult)
                    oagg = mp.tile([128, E], fp, name=f"oagg_{sfx}_{nb}",
                                   tag="oagg", bufs=3)
                    ee.tensor_tensor(out=tmp[:, :2 * E],
                                     in0=tmp[:, :2 * E],
                                     in1=tmp[:, 2 * E:4 * E], op=ALU.add)
                    ee.tensor_tensor(out=oagg[:], in0=tmp[:, :E],
                                     in1=tmp[:, E:2 * E], op=ALU.add)
                    out_t = mp.tile([128, E], fp, name=f"out_{sfx}_{nb}",
                                    tag="outt", bufs=3)
                    ee.tensor_tensor(out=out_t[:], in0=oagg[:],
                                     in1=src_sb[ph, nb][:], op=ALU.add)
                    nc.scalar.dma_start(out_dram[nb * 128:(nb + 1) * 128, :],
                                        out_t[:])
                    if payload is not None:
                        ag_in, ag_flat = payload
                        obf = mp.tile([128, E], bf, name=f"obf_{sfx}_{nb}",
                                      tag="obf", bufs=3)
                        ee.tensor_copy(obf[:], out_t[:])
                        nc.sync.dma_start(ag_in[nb * 128:(nb + 1) * 128, :],
                                          obf[:])
                        # pw = user_out @ Wp[1] + b, accumulated from srcT and
                        # the per-metapath aggT tiles against beta-scaled Wp
                        ppw2 = hp.tile([128, 512], fp, name=f"ppw2_{sfx}_{nb}",
                                       tag="h", space="PSUM")
                        for m in range(M):
                            psl2 = ppw2[:, m * D:(m + 1) * D]
                            nc.tensor.matmul(
                                psl2,
                                lhsT=srcT[ph][:, nb * 128:(nb + 1) * 128],
                                rhs=Wpb[1, m], start=True, stop=False)
                            for mp_ in range(M):
                                nc.tensor.matmul(
                                    psl2, lhsT=aggT_all[:, mp_ * E:(mp_ + 1) * E],
                                    rhs=wpsc[:, (mp_ * M + m) * D:
                                             (mp_ * M + m + 1) * D],
                                    start=False, stop=(mp_ == M - 1))
                        pwo = mp.tile([128, M * D], bf, name=f"pwo_{sfx}_{nb}",
                                      tag="pwo", bufs=3)
                        nc.vector.tensor_tensor(out=pwo[:], in0=ppw2[:, :M * D],
                                                in1=b_all[1][:], op=ALU.add)
                        dst = ag_flat[NLOC * E:].rearrange(
                            "(m n d) -> n m d", m=M,
                            d=D)[nb * 128:(nb + 1) * 128]
                        nc.sync.dma_start(
                            dst, pwo[:].rearrange("p (m d) -> p m d", m=M))

            for rep in range(REPEAT):
                ag_in = dp.tile([PAYR, E], bf, name=f"ag_in_{rep}")
                ag_out = dp.tile([CORES * PAYR, E], bf, name=f"ag_out_{rep}",
                                 addr_space=shared)
                agf = ag_out[:].rearrange("a b -> (a b)")
                t5bd = dp.tile([NBLK, K * D], bf, name=f"t5bd_{rep}")
                t6 = dp.tile([NBLK, K * E], bf, name=f"t6_{rep}")

                # ============= phase 1: users ============================
                t5a_j = t5a[:].rearrange("(m j) x -> j m x", m=M)
                pperm_r = t_pperm[:].rearrange("(m r) e -> r m e", m=M)

                def t5a_nb(nb):
                    return t5a_j[4 * nb:4 * nb + 4]

                def ge1_nb(nb):
                    return pperm_r[nb * 128:(nb + 1) * 128]

                emit_phase(0, f"{rep}a", t5a_nb, ge1_nb, t_uout,
                           (ag_in, ag_in[:].rearrange("a b -> (a b)")))
                if VARIANT == "tlprof":
                    nc.gpsimd.dma_start(ag_out[:PAYR, :], ag_in[:])
                else:
                    nc.gpsimd.collective_compute(
                        "AllGather", mybir.AluOpType.bypass,
                        replica_groups=[list(range(CORES))],
                        ins=[ag_in.opt()], outs=[ag_out.opt()])

                # phase-2 tables: one indirect reorder each into SBUF, then
                # a DRAM roundtrip for rearranged per-block static reads
                t5b = mp.tile([NBLK, K * D], bf, name=f"t5b_{rep}",
                              tag="t5b", bufs=2)
                nc.gpsimd.indirect_dma_start(
                    out=t5b[:], out_offset=None,
                    in_=agf.rearrange("(r x) -> r x", x=K * D),
                    in_offset=bass.IndirectOffsetOnAxis(ap=ipn2[:], axis=0))
                nc.scalar.dma_start(t5bd[:], t5b[:])
                t6sb = mp.tile([NBLK, K * E], bf, name=f"t6sb_{rep}",
                               tag="t6sb", bufs=2)
                nc.gpsimd.indirect_dma_start(
                    out=t6sb[:], out_offset=None,
                    in_=agf.rearrange("(r x) -> r x", x=K * E),
                    in_offset=bass.IndirectOffsetOnAxis(ap=ige2[:], axis=0))
                nc.sync.dma_start(t6[:], t6sb[:])

                # ============= phase 2: products =========================
                t5b_j = t5bd[:].rearrange("(m j) x -> j m x", m=M)
                t6_r = t6[:].rearrange("(m j) (u e) -> (j u) m e", m=M, e=E)

                def t5b_nb(nb):
                    return t5b_j[4 * nb:4 * nb + 4]

                def ge2_nb(nb):
                    return t6_r[nb * 128:(nb + 1) * 128]

                emit_phase(1, f"{rep}b", t5b_nb, ge2_nb, t_pout, None)

    nc.compile()
    return nc


def _get_graph():
    key = ("nc", VARIANT, REPEAT)
    if key not in _CACHE:
        _CACHE[key] = _build_graph()
    return _CACHE[key]


# ---------------------------------------------------------------- runner
def _get_runner():
    """Build (once) a cached jitted SPMD executable for the graph."""
    rkey = ("runner", VARIANT, REPEAT)
    if rkey in _CACHE:
        return _CACHE[rkey]
    import sys
    if "/opt/trn_rl_repo" not in sys.path:
        sys.path.insert(0, "/opt/trn_rl_repo")
    import jax
    import numpy as _np
    from jax.experimental.shard_map import shard_map
    from jax.sharding import Mesh, PartitionSpec
    from concourse import bass2jax, mybir

    nc = _get_graph()
    bass2jax.install_neuronx_cc_hook()
    assert nc.dbg_addr is None
    pid_name = nc.partition_id_tensor.name if nc.partition_id_tensor else None

    in_names, out_names, out_avals = [], [], []
    for alloc in nc.m.functions[0].allocations:
        if not isinstance(alloc, mybir.MemoryLocationSet):
            continue
        name = alloc.memorylocations[0].name
        if alloc.kind == "ExternalInput":
            if name != pid_name:
                in_names.append(name)
        elif alloc.kind == "ExternalOutput":
            out_names.append(name)
            out_avals.append(jax.core.ShapedArray(
                tuple(alloc.tensor_shape), mybir.dt.np(alloc.dtype)))
    n_params = len(in_names)
    all_names = in_names + out_names
    if pid_name is not None:
        all_names = all_names + [pid_name]

    def _body(*args):
        operands = list(args)
        if pid_name is not None:
            operands.append(bass2jax.partition_id_tensor())
        outs = bass2jax._bass_exec_p.bind(
            *operands, out_avals=tuple(out_avals), in_names=tuple(all_names),
            out_names=tuple(out_names), lowering_input_output_aliases=(),
            sim_require_finite=True, sim_require_nnan=True, nc=nc)
        return tuple(outs)

    devices = jax.devices()[:CORES]
    mesh = Mesh(_np.asarray(devices), ("core",))
    n_outs = len(out_names)
    in_specs = (PartitionSpec("core"),) * (n_params + n_outs)
    out_specs = (PartitionSpec("core"),) * n_outs
    donate = tuple(range(n_params, n_params + n_outs))
    sharded = jax.jit(
        shard_map(_body, mesh=mesh, in_specs=in_specs, out_specs=out_specs,
                  check_rep=False),
        donate_argnums=donate, keep_unused=True)

    runner = dict(fn=sharded, in_names=in_names, out_names=out_names,
                  out_avals=out_avals, mesh=mesh)
    _CACHE[rkey] = runner
    return runner


def _run_spmd(in_maps, timeit=0):
    """Run the SPMD graph; returns (per-core results list, best_step_ns|None)."""
    import jax
    import numpy as _np
    import time as _time
    from jax.sharding import NamedSharding, PartitionSpec

    r = _get_runner()
    fn, in_names, out_names, out_avals = \
        r["fn"], r["in_names"], r["out_names"], r["out_avals"]
    mesh = r["mesh"]

    concat_in = [_np.concatenate([_np.asarray(in_maps[c][k]) for c in range(CORES)],
                                 axis=0) for k in in_names]
    sharding = NamedSharding(mesh, PartitionSpec("core"))
    dev_in = [jax.device_put(a, sharding) for a in concat_in]

    def zeros():
        return [jax.device_put(
            _np.zeros((CORES * av.shape[0], *av.shape[1:]), av.dtype), sharding)
            for av in out_avals]

    outs = fn(*dev_in, *zeros())
    jax.block_until_ready(outs)
    best_ns = None
    if timeit:
        zs = [zeros() for _ in range(timeit)]
        for z in zs:
            jax.block_until_ready(z)
        t0 = _time.perf_counter()
        outs2 = fn(*dev_in, *zs[0])
        jax.block_until_ready(outs2)
        t_one = _time.perf_counter() - t0
        t0 = _time.perf_counter()
        many = [fn(*dev_in, *z) for z in zs[1:]]
        for o in many:
            jax.block_until_ready(o)
        t_many = _time.perf_counter() - t0
        per = t_many / (timeit - 1)
        best_ns = int(per * 1e9)
        print(f"[timing] single {t_one*1e3:.2f} ms, pipelined avg {per*1e3:.3f} ms")
        outs = many[-1]
    np_outs = [_np.asarray(o) for o in outs]
    results = [{name: np_outs[i].reshape(CORES, *out_avals[i].shape)[c]
                for i, name in enumerate(out_names)} for c in range(CORES)]
    return results, best_ns


def _make_in_maps(user, product, V, X, W_p, B_p, W_q, B_q, Q,
                  user_nbrs, product_nbrs):
    import ml_dtypes
    bfnp = ml_dtypes.bfloat16

    Xrep = np.ascontiguousarray(
        np.broadcast_to(X[:, :, 0, :][:, None, :, :], (2, 128, M, D))
        .reshape(2, 128, M * D)).astype(np.float32)
    Brep = np.ascontiguousarray(
        np.broadcast_to(B_p[:, None, :, :], (2, 128, M, D))
        .reshape(2, 128, M * D)).astype(np.float32)
    CB = np.array(
        [float((N_NODES - K) * np.exp(np.float32(-1e-9)))] +
        [float((N_NODES - K) * np.exp(np.float32(1.0) / N_NODES))] * (M - 1),
        np.float32)
    CBrep = np.ascontiguousarray(
        np.broadcast_to(CB[None, None, :], (2, 128, M))).astype(np.float32)

    user_s = user[_PERM]
    prod_s = product[_PERM]
    # neighbor class per (m, own-class) -- constant across u (checked)
    r1 = (user_nbrs[:, :STRIDE, 0] % STRIDE).astype(np.int64)     # [M, 256]
    r2 = (product_nbrs[:, :STRIDE, 0] % STRIDE).astype(np.int64)  # [M, 256]

    ind_col = np.repeat(np.eye(4, dtype=np.float32), 32, axis=1)  # [4, 128]
    ind4 = np.tile(ind_col, (1, M * NLOC // 128)).astype(bfnp)
    Ibc = np.tile(np.eye(D, dtype=np.float32), (1, K)).astype(bfnp)

    u_ar = np.arange(K)
    mm = np.arange(M)[:, None]
    in_maps = []
    for c in range(CORES):
        rows = slice(c * NLOC, (c + 1) * NLOC)
        q_own = QPC * c + np.arange(QPC)
        rc1 = r1[:, q_own]                                   # [M, 32]
        rc2 = r2[:, q_own]
        # phase-1 pre-gathered neighbor rows (m, j, u): sorted row r*K + u
        src_rows = (rc1[:, :, None] * K + u_ar[None, None, :]).reshape(-1)
        pperm = np.ascontiguousarray(prod_s[src_rows]).astype(bfnp)
        ppermT = np.ascontiguousarray(pperm.T)
        # phase-2 reorder indices into the AllGather result
        c2 = rc2 // QPC
        qq = rc2 % QPC
        i_pn2 = (c2 * (PAYR * E // (K * D)) + NLOC * E // (K * D)
                 + mm * QPC + qq).astype(np.int32).reshape(-1)
        i_ge2 = (c2 * (PAYR * E // (K * E)) + qq).astype(np.int32).reshape(-1)
        in_maps.append({
            "user_shard": user_s[rows],
            "product_shard": prod_s[rows],
            "userT": np.ascontiguousarray(user_s[rows].T.astype(bfnp)),
            "prodT": np.ascontiguousarray(prod_s[rows].T.astype(bfnp)),
            "prod_perm": pperm,
            "prod_permT": ppermT,
            "ind4": ind4,
            "Ibc": Ibc,
            "V_w": V, "Wp_w": W_p, "Wq_w": W_q,
            "Xrep": Xrep, "Brep": Brep, "CBrep": CBrep,
            "Bq_w": B_q, "Q_w": Q,
            "i_pn2": i_pn2, "i_ge2": i_ge2,
        })
    return in_maps


# ---------------------------------------------------------------- entry
def kernel(user, product, V, X, W_p, B_p, W_q, B_q, Q, user_nbrs, product_nbrs):
    user = np.asarray(user, np.float32)
    product = np.asarray(product, np.float32)
    V = np.asarray(V, np.float32)
    X = np.asarray(X, np.float32)
    W_p = np.asarray(W_p, np.float32)
    B_p = np.asarray(B_p, np.float32)
    W_q = np.asarray(W_q, np.float32)
    B_q = np.asarray(B_q, np.float32)
    Q = np.asarray(Q, np.float32)
    user_nbrs = np.asarray(user_nbrs)
    product_nbrs = np.asarray(product_nbrs)

    if not (_check_structured(user_nbrs) and _check_structured(product_nbrs)):
        # General-index fallback: same math on the host.
        return _reference_np(user, product, V, X, W_p, B_p, W_q, B_q, Q,
                             user_nbrs, product_nbrs)

    in_maps = _make_in_maps(user, product, V, X, W_p, B_p, W_q, B_q, Q,
                            user_nbrs, product_nbrs)
    results, _ = _run_spmd(in_maps)
    user_out = np.concatenate([results[c]["user_out_shard"]
                               for c in range(CORES)], axis=0)[_IPERM]
    product_out = np.concatenate([results[c]["product_out_shard"]
                                  for c in range(CORES)], axis=0)[_IPERM]
    return (user_out, product_out)
